# revision 33
# baseline (speedup 1.0000x reference)
"""Bezier Gaussian-splat raster kernel for 8 Trainium2 NeuronCores.

Problem: control_points [16,4,4,2] f32, sigma scalar f32 ->
raster [16,4,1,512,512] f32 where
  raster[b,s,0,p,q] = sum_t exp(-((y_t-g_p)^2+(x_t-g_q)^2)/(2 sigma^2))
with (x_t,y_t) the cubic Bezier curve sampled at 128 points and
g = arange(512)/512.

Strategy (data-parallel, no cross-core comms):
  - 16 batches / 8 cores -> 2 batches = 8 strokes per core.
  - Per stroke: Ax[t,q] = exp(-(x_t-g_q)^2 * inv) built in ONE scalar-engine
    op via Derivative_Erf(u) = 2/sqrt(pi) * exp(-u^2) with
    u = sinv*g - sinv*x  (sinv = 1/(sigma*sqrt(2)) as per-partition
    scale/bias APs, computed on device from the sigma input).
  - raster chunk = Ay[:,128p-chunk].T @ Ax on the tensor engine (fp16 in,
    fp32 PSUM out), 4 chunks per stroke.
  - PSUM->SBUF drain scaled by pi/4 (cancels the (2/sqrt(pi))^2), split
    3:1 between the vector and scalar engines.
  - One 256KiB HWDGE DMA per chunk to HBM (the steady-state bound).
"""

import math

import numpy as np

import concourse.bass as bass
import concourse.mybir as mybir
import concourse.tile as tile
from concourse import bacc
from concourse.bass_utils import run_bass_kernel_spmd

RES = 512
STEPS = 128
NK = 4            # control points per stroke
B_FULL = 16
S_FULL = 4
N_CORES = 8
BPC = B_FULL // N_CORES      # batches per core
SPC = BPC * S_FULL           # strokes per core
PCHUNKS = RES // 128         # 128-row chunks of the raster

F16 = mybir.dt.float16
F32 = mybir.dt.float32
AF = mybir.ActivationFunctionType

PI_OVER_4 = math.pi / 4.0
SQRT2 = math.sqrt(2.0)


def _bernstein() -> np.ndarray:
    t = np.linspace(0.0, 1.0, STEPS, dtype=np.float64)
    rows = [math.comb(NK - 1, k) * t ** (NK - 1 - k) * (1.0 - t) ** k
            for k in range(NK)]
    return np.stack(rows).astype(np.float32)  # [4, 128] = feat[k, t]


def build_bass(repeats: int = 1, probe: str = "") -> bass.Bass:
    """Build the per-core Bass program. `repeats` re-runs the whole stroke
    loop N times (same outputs) — used only by the timing harness to
    estimate steady-state per-iteration HW time from wall-clock deltas."""
    nc = bacc.Bacc("TRN2", target_bir_lowering=False, debug=False,
                   num_devices=N_CORES)

    # sig3 = [sigma, -1/sqrt2, +1/sqrt2]; cp_aug[:, :16] = control-point
    # coords (x strokes 0-7, y strokes 0-7).
    sig_in = nc.dram_tensor("sig3", [1, 3], F32, kind="ExternalInput")
    cp_in = nc.dram_tensor("cp_aug", [NK, 2 * SPC], F32,
                           kind="ExternalInput")
    out = nc.dram_tensor("out", [SPC, PCHUNKS, 128, RES], F32,
                         kind="ExternalOutput")

    g_rep_np = np.ascontiguousarray(np.broadcast_to(
        np.arange(RES, dtype=np.float32) / np.float32(RES), (128, RES)))
    g_dram = nc.inline_tensor(g_rep_np, "g_rep")
    feat_dram = nc.inline_tensor(_bernstein(), "feat")

    with tile.TileContext(nc) as tc:
        with tc.tile_pool(name="const", bufs=1) as cpool:
            # Warm the ACT table set (~2.7us load) immediately, overlapping
            # the setup chain: a dep-free Derivative_Erf on a memset tile.
            warm = cpool.tile([1, 1], F32)
            nc.gpsimd.memset(warm[:], 0.0)
            nc.scalar.activation(warm[:], warm[:], AF.Derivative_Erf,
                                 bias=0.0, scale=0.0)

            sig_t = cpool.tile([1, 3], F32)
            nc.sync.dma_start(sig_t[:], sig_in[:])
            g_tile = cpool.tile([128, RES], F32)
            nc.scalar.dma_start(g_tile[:], g_dram[:])
            cp_t = cpool.tile([NK, 2 * SPC], F32)
            nc.sync.dma_start(cp_t[:], cp_in[:])
            feat_tile = cpool.tile([NK, STEPS], F32)
            nc.scalar.dma_start(feat_tile[:], feat_dram[:])
            g_rep = g_tile[:]
            feat_t = feat_tile[:]
            ones_t = cpool.tile([1, 128], F32)
            nc.vector.memset(ones_t[:], 1.0)

            # sinv = 1/(sigma*sqrt(2)): recip(sigma) * [-1/sqrt2, +1/sqrt2]
            s1 = cpool.tile([1, 1], F32)
            nc.vector.reciprocal(s1[:], sig_t[0:1, 0:1])
            pm = cpool.tile([1, 2], F32)
            nc.vector.tensor_scalar(pm[:], sig_t[0:1, 1:3], s1[:, 0:1], None,
                                    mybir.AluOpType.mult)

            sinv_sb = cpool.tile([128, 2], F32)   # col0=-sinv col1=+sinv
            bias_sb = cpool.tile([128, 2 * SPC], F32)  # -sinv * xy_j(t)
            with tc.tile_pool(name="spsum", bufs=1, space="PSUM") as spool:
                pbc = spool.tile([128, 2], F32)
                nc.tensor.matmul(pbc[:], lhsT=ones_t[:], rhs=pm[:])
                nc.vector.tensor_copy(sinv_sb[:], pbc[:])

                # raw-cp bias matmul runs parallel to the sinv chain; the
                # -sinv scaling happens in the PSUM->SBUF copy.
                bps = spool.tile([128, 2 * SPC], F32)
                nc.tensor.matmul(bps[:], lhsT=feat_t, rhs=cp_t[:])
                nc.vector.tensor_scalar(bias_sb[:], bps[:],
                                        sinv_sb[:, 0:1], None,
                                        mybir.AluOpType.mult)

            if "dmaonly" in probe:
                dsrc = cpool.tile([128, PCHUNKS * RES], F32)
                nc.vector.memset(dsrc[:], 0.25)
                for s in [s for _ in range(repeats) for s in range(SPC)]:
                    if "big" in probe:
                        dst = out[s].rearrange("c p q -> p c q")
                        src = dsrc[:].rearrange("p (c q) -> p c q", c=PCHUNKS)
                        nc.sync.dma_start(dst, src)
                    else:
                        for c in range(PCHUNKS):
                            nc.sync.dma_start(out[s, c],
                                              dsrc[:, c * RES:(c + 1) * RES])
                stroke_iters = []
            else:
                stroke_iters = [s for _ in range(repeats) for s in range(SPC)]

            with tc.tile_pool(name="apool", bufs=6) as apool, \
                 tc.tile_pool(name="opool", bufs=12) as opool, \
                 tc.tile_pool(name="mmpool", bufs=8, space="PSUM") as mmpool:
                for s in stroke_iters:
                    ax = apool.tile([128, RES], F16, tag="ax")
                    nc.scalar.activation(ax[:], g_rep, AF.Derivative_Erf,
                                         bias=bias_sb[:, s:s + 1],
                                         scale=sinv_sb[:, 1:2])
                    ay = apool.tile([128, RES], F16, tag="ay")
                    nc.scalar.activation(ay[:], g_rep, AF.Derivative_Erf,
                                         bias=bias_sb[:, SPC + s:SPC + s + 1],
                                         scale=sinv_sb[:, 1:2])
                    for c in range(PCHUNKS):
                        if "nomm" not in probe:
                            ps = mmpool.tile([128, RES], F32, tag="ps")
                            nc.tensor.matmul(ps[:],
                                             lhsT=ay[:, c * 128:(c + 1) * 128],
                                             rhs=ax[:])
                        if "nocopy" not in probe:
                            ot = opool.tile([128, RES], F32, tag="ot")
                            # balance PSUM->SBUF drain across DVE and ACT
                            if c == 3:
                                nc.scalar.mul(ot[:], ps[:], PI_OVER_4)
                            else:
                                nc.vector.tensor_scalar_mul(ot[:], ps[:],
                                                            PI_OVER_4)
                        if "nodma" not in probe:
                            eng = nc.scalar if ("actdma" in probe and
                                                c % 2 == 1) else nc.sync
                            eng.dma_start(out[s, c], ot[:])

    nc.finalize()
    return nc


_CACHE: dict = {}


def _get_nc() -> bass.Bass:
    if "nc" not in _CACHE:
        _CACHE["nc"] = build_bass()
    return _CACHE["nc"]


def _in_maps(control_points: np.ndarray, sigma) -> list:
    cp = np.asarray(control_points, dtype=np.float32)
    sig = np.float32(np.asarray(sigma).reshape(()))
    isq2 = np.float32(1.0 / SQRT2)
    sig3 = np.array([[sig, -isq2, isq2]], dtype=np.float32)
    maps = []
    for c in range(N_CORES):
        cpc = cp[BPC * c:BPC * (c + 1)].reshape(SPC, NK, 2)
        cp_aug = np.concatenate([cpc[:, :, 0].T, cpc[:, :, 1].T], axis=1)
        maps.append({"sig3": sig3.copy(),
                     "cp_aug": np.ascontiguousarray(cp_aug)})
    return maps


def run(control_points, sigma, **spmd_kwargs):
    """Run on HW; returns (full_output, BassKernelResults)."""
    nc = _get_nc()
    res = run_bass_kernel_spmd(nc, _in_maps(control_points, sigma),
                               core_ids=list(range(N_CORES)), **spmd_kwargs)
    outs = [r["out"].reshape(BPC, S_FULL, RES, RES) for r in res.results]
    full = np.concatenate(outs, axis=0)[:, :, None]
    return np.ascontiguousarray(full, dtype=np.float32), res


def kernel(control_points, sigma):
    return run(control_points, sigma)[0]
